# revision 12
# baseline (speedup 1.0000x reference)
"""LoRA MLP (2->64->64->64->64->64->3, tanh) over N=1,048,576 rows.

Strategy:
  - Host: merge LoRA into dense weights (W_eff = W + B@A), build
    block-diagonal lhsT so each 128-wide PE pass processes TWO row-chunks
    (features of chunk A on partitions 0..63, chunk B on 64..127).
  - 8 cores, pure data parallel: 131072 rows/core = 65536 columns
    (each SBUF column carries one row of chunk A and one row of chunk B).
  - Per 2048-col block: 4 fp16 matmuls (full-rate on the PE) into a
    4-bank fp32 PSUM tile, one [128,2048] ACT tanh with fused fp32
    per-partition bias, last layer bias-add on DVE. tanh on the scalar
    engine is the bottleneck; PE/DVE/DMA hide underneath it.
  - fp16 end-to-end numerics emulated on host: max scale-relative error
    ~1.1e-3 vs the fp32 reference (bf16 would be ~8e-3).
  - x stays fully SBUF-resident (one DMA); outputs stream back per block.
"""

import numpy as np
from contextlib import ExitStack

import concourse.bacc as bacc
import concourse.tile as tile
from concourse import mybir
from concourse.bass_utils import run_bass_kernel_spmd

N = 1_048_576
NCORES = 8
N_CORE = N // NCORES          # 131072 rows per core
NCOLS = N_CORE // 2           # 65536 cols (2 rows per col: chunk A + chunk B)
BLK = 2048                    # columns per block (PSUM tile = 4 banks)
NBLK = NCOLS // BLK           # 32 blocks
MM = 512                      # moving free dim per matmul (1 PSUM bank)
WB_COLS = 646                 # packed fp16 weight tensor columns

F32 = mybir.dt.float32
F16 = mybir.dt.float16

# Set by the last kernel() call (profiling info for test.py).
LAST_RESULT = None


def build_nc(repeat=1):
    nc = bacc.Bacc(None, target_bir_lowering=False)

    xt = nc.dram_tensor("xt", [4, NCOLS], F16, kind="ExternalInput")
    wb = nc.dram_tensor("wb", [128, WB_COLS], F16, kind="ExternalInput")
    bias = nc.dram_tensor("bias", [128, 6], F32, kind="ExternalInput")
    out_t = nc.dram_tensor("out_t", [6, NCOLS], F16, kind="ExternalOutput")

    with tile.TileContext(nc) as tc, ExitStack() as ctx:
        const = ctx.enter_context(tc.tile_pool(name="const", bufs=1))
        h_pool = ctx.enter_context(tc.tile_pool(name="h", bufs=6))
        o_pool = ctx.enter_context(tc.tile_pool(name="o", bufs=3))
        ps_pool = ctx.enter_context(tc.tile_pool(name="ps", bufs=2, space="PSUM"))

        wb_sb = const.tile([128, WB_COLS], F16, tag="wb")
        nc.gpsimd.dma_start(out=wb_sb, in_=wb[:, :])
        bias_sb = const.tile([128, 6], F32, tag="bias")
        nc.gpsimd.dma_start(out=bias_sb, in_=bias[:, :])

        # whole per-core x resident in SBUF: one DMA, no slot reuse
        xfull = const.tile([4, NCOLS], F16, tag="xfull")
        nc.gpsimd.dma_start(out=xfull, in_=xt[:, :])

        # lhsT views: layer1 [4,128] at cols 512..639 (rows 0..3),
        # layers 2..5 [128,128] at cols 0..511, layer6 [128,6] at 640..645
        w_sb = [wb_sb[0:4, 512:640]]
        for i in range(4):
            w_sb.append(wb_sb[:, i * 128 : (i + 1) * 128])
        w_sb.append(wb_sb[:, 640:646])
        b_sb = [bias_sb[:, i : i + 1] for i in range(5)]
        b_sb.append(bias_sb[0:6, 5:6])

        for blk in range(NBLK * repeat):
            c0 = (blk % NBLK) * BLK
            h = xfull[:, c0 : c0 + BLK]
            for layer in range(6):
                ps = ps_pool.tile([128, BLK], F32, tag="ps")
                po = ps[:, :] if layer < 5 else ps[0:6, :]
                for q in range(BLK // MM):
                    nc.tensor.matmul(
                        out=po[:, q * MM : (q + 1) * MM],
                        lhsT=w_sb[layer],
                        rhs=h[:, q * MM : (q + 1) * MM],
                    )
                if layer < 5:
                    hn = h_pool.tile([128, BLK], F16, tag="h")
                    nc.scalar.activation(
                        out=hn[:, :],
                        in_=ps[:, :],
                        func=mybir.ActivationFunctionType.Tanh,
                        bias=b_sb[layer],
                    )
                    h = hn

            # layer 6 epilogue: bias add on DVE, then DMA out
            ot = o_pool.tile([6, BLK], F16, tag="o")
            nc.vector.tensor_scalar_add(ot[:, :], ps[0:6, :], b_sb[5])
            nc.gpsimd.dma_start(out=out_t[:, c0 : c0 + BLK], in_=ot)

    nc.compile()
    return nc


def _prep_weights(inputs):
    """Merged LoRA weights (fp16, block-diagonal lhsT) + fp32 biases."""

    def eff(w, bmat, amat):
        return (
            w.astype(np.float64) + bmat.astype(np.float64) @ amat.astype(np.float64)
        ).astype(np.float32)

    wb = np.zeros((128, WB_COLS), np.float16)
    for i in (2, 3, 4, 5):
        wl = eff(inputs[f"W{i}"], inputs[f"B{i}"], inputs[f"A{i}"])  # [64, 64]
        c = (i - 2) * 128
        wb[0:64, c : c + 64] = wl.T.astype(np.float16)
        wb[64:128, c + 64 : c + 128] = wl.T.astype(np.float16)
    w1 = eff(inputs["W1"], inputs["B1"], inputs["A1"])  # [64, 2]
    wb[0:2, 512:576] = w1.T.astype(np.float16)
    wb[2:4, 576:640] = w1.T.astype(np.float16)
    w6 = eff(inputs["W6"], inputs["B6"], inputs["A6"])  # [3, 64]
    wb[0:64, 640:643] = w6.T.astype(np.float16)
    wb[64:128, 643:646] = w6.T.astype(np.float16)

    bias = np.zeros((128, 6), np.float32)
    for i in (1, 2, 3, 4, 5):
        b = np.asarray(inputs[f"b{i}"], np.float32).reshape(64)
        bias[:, i - 1] = np.concatenate([b, b])
    b6 = np.asarray(inputs["b6"], np.float32).reshape(3)
    bias[0:3, 5] = b6
    bias[3:6, 5] = b6
    return {"wb": wb, "bias": bias}


def kernel(**inputs):
    global LAST_RESULT
    inputs = {k: np.asarray(v, np.float32) for k, v in inputs.items()}
    ws = _prep_weights(inputs)

    x = inputs["x"]  # [N, 2]
    in_maps = []
    for c in range(NCORES):
        sh = x[c * N_CORE : (c + 1) * N_CORE]  # [131072, 2]
        xtc = np.empty((4, NCOLS), np.float16)
        xtc[0:2] = sh[:NCOLS].T
        xtc[2:4] = sh[NCOLS:].T
        m = {"xt": np.ascontiguousarray(xtc)}
        m.update(ws)
        in_maps.append(m)

    nc = build_nc()
    res = run_bass_kernel_spmd(nc, in_maps, core_ids=list(range(NCORES)))
    LAST_RESULT = res

    u = np.empty((N, 1), np.float32)
    v = np.empty((N, 1), np.float32)
    w = np.empty((N, 1), np.float32)
    for c in range(NCORES):
        o = res.results[c]["out_t"]  # [6, NCOLS] fp16
        base = c * N_CORE
        u[base : base + NCOLS, 0] = o[0]
        v[base : base + NCOLS, 0] = o[1]
        w[base : base + NCOLS, 0] = o[2]
        u[base + NCOLS : base + N_CORE, 0] = o[3]
        v[base + NCOLS : base + N_CORE, 0] = o[4]
        w[base + NCOLS : base + N_CORE, 0] = o[5]
    return (u, v, w)
